# revision 15
# baseline (speedup 1.0000x reference)
"""Trainium2 Bass kernel for nn_Conditioning (embedding lookup + concat).

Reference computation:
    gc = W.T[ids] + b          # (B, T, 64) gather from a tiny 128x64 table
    out = concat(lc, gc, -1)   # (B, T, 128)

Shapes: lc (16, 32768, 64) f32, ids (16, 32768) int64, W (64, 128) f32,
b (64,) f32 -> out (16, 32768, 128) f32.

Sharding: data-parallel over batch — 2 batches (65536 tokens) per core on
8 cores; W and b replicated.

Device algorithm (per core), memory-roofline oriented (~48 MB HBM traffic
= ~134 us at 358 GB/s/core):
  * One-time: build WTb = W.T + b in SBUF (bias broadcast across partitions
    via GpSimd partition_broadcast), then split into a packed bf16 table
    wtbx = [bf16(WTb) | bf16(WTb - bf16(WTb))] (hi|lo halves) so the gather
    is exact to ~2^-16 relative after the hi+lo re-add; iota column
    (partition index, f32) for one-hot building.
  * Per macro-tile of 128*Q tokens (token t = Q*p + q <-> partition p,
    slot q; Q=32 steady state, with a short Q=8 ramp-up prologue so the
    first stores issue early and the DMA engines never idle):
      - DMA ids row (1, 128*Q) bf16 (ScalarE HWDGE);
        GpSimd partition_broadcast -> (128, 128*Q) bf16.
      - VectorE is_equal(ids_bcast, iota) -> one-hot (speaker, token) bf16.
      - Q matmuls (one-hot (128,128) stationary, packed wtbx (128,128)
        moving) -> PSUM (token-slot, [hi64|lo64]) f32, 8 slots per PSUM
        tile (2 banks, 4 bufs).
      - hi half copied into the gc columns of the assembled out tile
        (ScalarE/VectorE alternating); VectorE adds the lo PSUM half in
        place (exact f32 re-add, one PSUM operand per op).
      - DMA lc into a contiguous staging tile (Sync HWDGE); ScalarE
        copies it into the interleaved lc columns of the out tile.
      - One fully contiguous store per macro (Sync HWDGE, 2 MB steady
        state).
"""

import sys

for _p in ("/opt/trn_rl_repo",):
    if _p not in sys.path:
        sys.path.insert(0, _p)

from contextlib import ExitStack

import ml_dtypes
import numpy as np

import concourse.bass as bass  # noqa: F401
import concourse.tile as tile
from concourse import bacc, mybir
from concourse.bass_utils import run_bass_kernel_spmd

N_CORES = 8
B, T, I = 16, 32768, 64
N_SPK, N_EMBED = 128, 64
P = 128  # partitions
TOK_PER_CORE = B * T // N_CORES  # 65536
# (tokens-per-partition Q, macro count): short ramp-up then 4096-token macros
SCHEDULE = ((8, 4), (32, 15))
CHUNK = 8  # psum rotation granularity (8 slots = 2 banks, 4 bufs)

F32 = mybir.dt.float32
BF16 = mybir.dt.bfloat16

assert sum(P * q * c for q, c in SCHEDULE) == TOK_PER_CORE


def _macro_list(schedule):
    tok0, out = 0, []
    for q, cnt in schedule:
        for _ in range(cnt):
            out.append((tok0, q))
            tok0 += P * q
    return out, tok0


def build_bass(schedule=SCHEDULE):
    macros, tok = _macro_list(schedule)
    max_q = max(q for _, q in macros)

    nc = bacc.Bacc("TRN2", target_bir_lowering=False, debug=False)
    lc = nc.dram_tensor("lc", (tok, I), F32, kind="ExternalInput").ap()
    ids = nc.dram_tensor("ids", (tok,), BF16, kind="ExternalInput").ap()
    wt = nc.dram_tensor("wt", (N_SPK, N_EMBED), F32, kind="ExternalInput").ap()
    bi = nc.dram_tensor("bias", (1, N_EMBED), F32, kind="ExternalInput").ap()
    out = nc.dram_tensor("out", (tok, I + N_EMBED), F32, kind="ExternalOutput").ap()

    with tile.TileContext(nc) as tc, ExitStack() as ctx:
        const = ctx.enter_context(tc.tile_pool(name="const", bufs=1))
        ids_pool = ctx.enter_context(tc.tile_pool(name="idsrow", bufs=3))
        bc_pool = ctx.enter_context(tc.tile_pool(name="idsbc", bufs=2))
        oh_pool = ctx.enter_context(tc.tile_pool(name="onehot", bufs=2))
        lc_pool = ctx.enter_context(tc.tile_pool(name="lct", bufs=5))
        out_pool = ctx.enter_context(tc.tile_pool(name="outt", bufs=3))
        pgc_pool = ctx.enter_context(tc.tile_pool(name="pgc", bufs=4, space="PSUM"))

        # ---- one-time constants ----
        wt_sb = const.tile([N_SPK, N_EMBED], F32)
        nc.sync.dma_start(out=wt_sb[:], in_=wt[:])
        b_row = const.tile([1, N_EMBED], F32)
        nc.sync.dma_start(out=b_row[:], in_=bi[:])
        b_bc = const.tile([N_SPK, N_EMBED], F32)
        nc.gpsimd.partition_broadcast(b_bc[:], b_row[:])
        wtb = const.tile([N_SPK, N_EMBED], F32)
        nc.vector.tensor_tensor(
            out=wtb[:], in0=wt_sb[:], in1=b_bc[:], op=mybir.AluOpType.add
        )
        # packed bf16 table: [hi | lo]
        wtbx = const.tile([N_SPK, 2 * N_EMBED], BF16)
        nc.vector.tensor_copy(out=wtbx[:, 0:N_EMBED], in_=wtb[:])
        hi_f32 = const.tile([N_SPK, N_EMBED], F32)
        nc.vector.tensor_copy(out=hi_f32[:], in_=wtbx[:, 0:N_EMBED])
        nc.vector.tensor_tensor(
            out=wtbx[:, N_EMBED : 2 * N_EMBED],
            in0=wtb[:],
            in1=hi_f32[:],
            op=mybir.AluOpType.subtract,
        )
        iota_i = const.tile([P, 1], mybir.dt.int32)
        nc.gpsimd.iota(iota_i[:], pattern=[[0, 1]], base=0, channel_multiplier=1)
        iota_f = const.tile([P, 1], F32)
        nc.vector.tensor_copy(out=iota_f[:], in_=iota_i[:])

        # ---- main loop ----
        for tok0, q in macros:
            macro = P * q
            lc_re = lc[tok0 : tok0 + macro, :].rearrange("(p q) d -> p (q d)", p=P, q=q)
            out_re = out[tok0 : tok0 + macro, :].rearrange(
                "(p q) d -> p (q d)", p=P, q=q
            )
            ids_re = ids[tok0 : tok0 + macro].rearrange("(o m) -> o m", o=1)

            ids_row = ids_pool.tile([1, macro], BF16, tag="ids_row")
            nc.scalar.dma_start(out=ids_row[:], in_=ids_re)
            ids_bc = bc_pool.tile([P, macro], BF16, tag="ids_bc")
            nc.gpsimd.partition_broadcast(ids_bc[:], ids_row[:])
            onehot = oh_pool.tile([P, macro], BF16, tag="onehot")
            nc.vector.tensor_scalar(
                out=onehot[:],
                in0=ids_bc[:],
                scalar1=iota_f[:],
                scalar2=None,
                op0=mybir.AluOpType.is_equal,
            )

            lc_t = lc_pool.tile([P, q * I], F32, tag="lc_t")
            nc.sync.dma_start(out=lc_t[:], in_=lc_re)

            out_t = out_pool.tile([P, q, I + N_EMBED], F32, tag="out_t")
            chunk = min(CHUNK, q)
            for h in range(q // chunk):
                sl = slice(h * chunk, (h + 1) * chunk)
                psum_gc = pgc_pool.tile([P, chunk, 2 * N_EMBED], F32, tag="psum_gc")
                for jj in range(chunk):
                    j = h * chunk + jj
                    nc.tensor.matmul(
                        psum_gc[:, jj, :],
                        lhsT=onehot[:, j * P : (j + 1) * P],
                        rhs=wtbx[:],
                        start=True,
                        stop=True,
                    )
                # hi half -> out tile on ScalarE (keeps VectorE's per-macro
                # stream + pipeline drains well under the DMA-paced budget),
                # then the lo half is added in place — exact f32 re-add with
                # a single PSUM operand per DVE op
                nc.scalar.copy(
                    out_t[:, sl, I : I + N_EMBED], psum_gc[:, :, 0:N_EMBED]
                )
                nc.vector.tensor_tensor(
                    out=out_t[:, sl, I : I + N_EMBED],
                    in0=psum_gc[:, :, N_EMBED : 2 * N_EMBED],
                    in1=out_t[:, sl, I : I + N_EMBED],
                    op=mybir.AluOpType.add,
                )
            # interleave lc into the out tile
            nc.scalar.copy(out_t[:, :, 0:I], lc_t[:])
            nc.sync.dma_start(out=out_re, in_=out_t[:])

    nc.compile()
    return nc


_NC_CACHE: dict = {}


def _get_nc(schedule=SCHEDULE):
    if schedule not in _NC_CACHE:
        _NC_CACHE[schedule] = build_bass(schedule)
    return _NC_CACHE[schedule]


def prep_ids(ids_shard_flat, schedule=SCHEDULE):
    """bf16-encode and slot-group a per-core flat ids shard.

    Within each macro of 128*q tokens, token t = q*p + s must appear at
    column s*128 + p so that matmul group s's one-hot columns line up with
    PSUM slot p (pure layout permutation; values unchanged).
    """
    a = np.asarray(ids_shard_flat).astype(np.float32).astype(ml_dtypes.bfloat16)
    macros, tok = _macro_list(schedule)
    assert a.shape == (tok,)
    parts = []
    for tok0, q in macros:
        parts.append(a[tok0 : tok0 + P * q].reshape(P, q).T.reshape(-1))
    return np.ascontiguousarray(np.concatenate(parts))


def make_in_maps(lc, ids, W, b):
    """Shard full inputs into per-core input maps for the bass kernel."""
    lc_flat = np.ascontiguousarray(np.asarray(lc, dtype=np.float32)).reshape(B * T, I)
    ids_flat = np.asarray(ids).reshape(B * T)
    wt = np.ascontiguousarray(np.asarray(W, dtype=np.float32).T)  # (128, 64)
    bi = np.asarray(b, dtype=np.float32).reshape(1, N_EMBED)
    in_maps = []
    for c in range(N_CORES):
        s = slice(c * TOK_PER_CORE, (c + 1) * TOK_PER_CORE)
        in_maps.append(
            {
                "lc": lc_flat[s],
                "ids": prep_ids(ids_flat[s]),
                "wt": wt,
                "bias": bi,
            }
        )
    return in_maps


def run(lc, ids, W, b, trace: bool = False):
    """Run on 8 NeuronCores; returns (full_output, BassKernelResults)."""
    nc = _get_nc()
    in_maps = make_in_maps(lc, ids, W, b)
    res = run_bass_kernel_spmd(nc, in_maps, list(range(N_CORES)), trace=trace)
    out = np.concatenate(
        [res.results[c]["out"] for c in range(N_CORES)], axis=0
    ).reshape(B, T, I + N_EMBED)
    return np.ascontiguousarray(out, dtype=np.float32), res


def kernel(lc, ids, W, b):
    out, _ = run(lc, ids, W, b)
    return out


if __name__ == "__main__":
    rng = np.random.default_rng(0)
    lc = rng.standard_normal((B, T, I), dtype=np.float32)
    ids = rng.integers(0, N_SPK, size=(B, T), dtype=np.int64)
    W = rng.standard_normal((N_EMBED, N_SPK), dtype=np.float32)
    b = rng.standard_normal((N_EMBED,), dtype=np.float32)
    out = kernel(lc=lc, ids=ids, W=W, b=b)
    exp = np.concatenate((lc, W.T[ids] + b), axis=2)
    err = np.max(np.abs(out - exp)) / np.max(np.abs(exp))
    print("max abs rel-to-scale err:", err)
